# revision 43
# baseline (speedup 1.0000x reference)
"""MoE routing kernel for Trainium2 (8 NeuronCores, data-parallel over batch).

Reference computation (B=1024, PHASE=64, GATE=128, K=8, D=512):
    coeff = softmax(gateMLP(phase))                       # [B, K]
    per layer l in 0..2:
        y = sum_k coeff[:,k] * (y @ W_l[k]) + coeff @ b_l # [B, D]
        y = elu(y)  (layers 0,1 only)

Device mapping (per core, B_local = 128 rows):
  - Gate runs in transposed-activation layout (h.T = [g, b]); phase.T is
    prepared on the host, so no transposes are needed anywhere.
  - Activations carry a +1 shift: y' = elu(y)+1, with the shift absorbed
    into the next layer's bias (b' = b - W.sum(axis=in)) on the host. This
    makes the activation 3 ops: e=Exp(x), r=Relu(x), out=(e min 1)+r.
  - Softmax normalization is deferred: the expert path uses unnormalized
    e = exp(logits - max); 1/sum(e) is folded into the per-partition `scale`
    of the activation/copy that drains the layer's PSUM.
  - Layer 0 runs gate-independent per-expert matmuls (x.T is packed by the
    host) and post-scales with diag(e_k), so expert compute starts as soon
    as each 512KB weight chunk lands — fully overlapping the gate. PE
    emission is hand-interleaved (the engine executes in order).
  - Layers 1-2 pre-scale: z_k.T = y.T * e[:,k] via y_chunk.T @ diag(e_k)
    with 4 experts' diagonals concatenated per N=512 fp16 matmul — fusing
    transpose and per-sample scaling; then 32 accumulating matmuls + one
    bias matmul (contraction K=8) per layer into one PSUM bank.
  - All expert-path operands are fp16 (PSUM accumulates fp32): full TensorE
    rate and half the weight DMA, which is the binding roofline
    (12.6 MB/core at ~400 GB/s ~= 32 us of HBM stream).
"""

import numpy as np

import concourse.mybir as mybir
import concourse.tile as tile
from concourse import bacc

AFT = mybir.ActivationFunctionType
ALU = mybir.AluOpType
F32 = mybir.dt.float32
F16 = mybir.dt.float16
AX = mybir.AxisListType

B, PHASE, GATE, K, D = 1024, 64, 128, 8, 512
NCORES = 8
BL = B // NCORES          # 128 rows per core
IC = D // 128             # 4 contraction chunks of 128
LW = K * IC * D           # weight columns per layer (16384)


def emit_moe(tc, out_ap, ins):
    """Emit the per-core MoE program. ins is a dict of DRAM APs."""
    nc = tc.nc

    with (
        tc.tile_pool(name="consts", bufs=1) as cpool,
        tc.tile_pool(name="ypool", bufs=2) as ypool,
        tc.tile_pool(name="zpool", bufs=2) as zpool,
        tc.tile_pool(name="tmp", bufs=3) as tpool,
        tc.tile_pool(name="ps_out", bufs=2, space="PSUM") as ps_out,
        tc.tile_pool(name="ps_z", bufs=3, space="PSUM") as ps_z,
        tc.tile_pool(name="ps_exp", bufs=3, space="PSUM") as ps_exp,
    ):
        # ---- gate/const loads on the Activation HWDGE queue ----
        t_c32 = cpool.tile([128, 530], F32)
        nc.scalar.dma_start(out=t_c32, in_=ins["c32"])
        t_ident = t_c32[:, 0:128]
        t_gw1 = t_c32[:, 128:256]
        t_gw0 = t_c32[0:PHASE, 256:384]
        t_phT = t_c32[0:PHASE, 384:512]
        t_gw2 = t_c32[:, 512:520]
        t_gb0 = t_c32[:, 520:521]
        t_gb1 = t_c32[:, 521:522]
        t_gb2 = t_c32[0:1, 522:530]

        t_c16 = cpool.tile([128, D], F16)
        nc.scalar.dma_start(out=t_c16, in_=ins["c16"])
        t_xT = t_c16[:, 0:D]              # x.T chunks: [p, ic*128+b] = x[b, ic*128+p]

        t_cb = cpool.tile([K, 3 * D], F16)
        nc.scalar.dma_start(out=t_cb, in_=ins["cb"])
        t_bias = t_cb[0:K, 0:3 * D]

        # ---- expert weights: all 3 layers resident, per-(l,k) DMAs so each
        # expert's matmuls start as soon as its 512KB chunk lands.  One queue:
        # the per-core HBM share (~400GB/s) is the cap, a second queue only
        # degrades arrival order ----------------------------------------
        t_w = cpool.tile([128, 3 * LW], F16)
        for l in range(3):
            for k in range(K):
                nc.sync.dma_start(
                    out=t_w[:, l * LW + k * 2048:l * LW + (k + 1) * 2048],
                    in_=ins["W"][l, k],
                )

        # ACT warmup: pull the activation tables off the critical path.
        t_ones = cpool.tile([1, GATE], F32)
        nc.vector.memset(t_ones, 1.0)
        t_warm = tpool.tile([1, 8], F32, tag="warm")
        nc.scalar.activation(t_warm, t_ones[:, :8], AFT.Exp)
        t_warm2 = tpool.tile([1, 8], F32, tag="warm")
        nc.scalar.activation(t_warm2, t_ones[:, :8], AFT.Relu)
        # PE warmup: absorb the first-matmul pipeline fill during the DMA
        # latency window instead of on the gate's first matmul
        p_warm = ps_z.tile([128, 512], F32, tag="zps")
        nc.tensor.matmul(
            p_warm[:1, :8], lhsT=t_ones[:, 0:1], rhs=t_ones[:, 0:8],
            start=True, stop=True,
        )

        # ---- gate + layer-0, with PE emission interleaved --------------
        # PE executes in order, so layer-0's gate-independent per-expert
        # matmuls (x.T @ W0[k], paced by W-chunk arrival) are woven between
        # the gate's latency-bound matmuls.
        t_pe = zpool.tile([128, K * D], F16, tag="z")
        p_es = {}

        def expert_mms(k):
            p_e = ps_exp.tile([128, 512], F32, tag="pexp", name=f"p_e{k}")
            for ic in range(IC):
                nc.tensor.matmul(
                    p_e,
                    lhsT=t_xT[:, ic * 128:(ic + 1) * 128],
                    rhs=t_w[:, k * 2048 + ic * 512:k * 2048 + (ic + 1) * 512],
                    start=(ic == 0),
                    stop=(ic == 3),
                )
            p_es[k] = p_e

        def expert_copy(k):
            dst = t_pe[:, k * 512:(k + 1) * 512]
            if k % 2 == 0:
                nc.vector.tensor_copy(out=dst, in_=p_es[k])
            else:
                nc.scalar.copy(dst, p_es[k])

        # The whole gate runs inside the first W-chunk's DMA latency window,
        # so no expert interleaving: experts can't start before W0[0] lands
        # anyway, and an early gate means e/diag/eT are ready before the
        # first expert drain.
        p_g = ps_z.tile([128, 512], F32, tag="zps")
        nc.tensor.matmul(p_g[:GATE, :BL], lhsT=t_gw0, rhs=t_phT, start=True, stop=True)
        h1 = tpool.tile([GATE, BL], F32, tag="h")
        _elu1(nc, tpool, h1, p_g[:GATE, :BL], bias=t_gb0)

        p_g2 = ps_z.tile([128, 512], F32, tag="zps")
        nc.tensor.matmul(p_g2[:GATE, :BL], lhsT=t_gw1, rhs=h1, start=True, stop=True)
        h2 = tpool.tile([GATE, BL], F32, tag="h")
        _elu1(nc, tpool, h2, p_g2[:GATE, :BL], bias=t_gb1)

        # logits[b, k] (normal layout; gb2 via ones-row matmul)
        p_lg = ps_z.tile([128, 512], F32, tag="zps")
        nc.tensor.matmul(p_lg[:BL, :K], lhsT=h2, rhs=t_gw2, start=True, stop=False)
        nc.tensor.matmul(p_lg[:BL, :K], lhsT=t_ones, rhs=t_gb2, start=False, stop=True)

        # e = exp(logits - rowmax)   (unnormalized softmax numerator)
        t_nmx = tpool.tile([BL, 1], F32)
        nc.vector.reduce_max(t_nmx, p_lg[:BL, :K], axis=AX.X, negate=True)
        t_e = cpool.tile([BL, K], F32)
        nc.scalar.activation(t_e, p_lg[:BL, :K], AFT.Exp, bias=t_nmx, scale=1.0)

        # normalizer 1/sum(e) — consumed much later as a PSUM-drain scale
        t_sum = tpool.tile([BL, 1], F32)
        nc.vector.reduce_sum(t_sum, t_e, axis=AX.X)
        t_rcp = cpool.tile([BL, 1], F32)
        nc.vector.reciprocal(t_rcp, t_sum)

        # e.T (fp16) for the mixed-bias matmul
        p_et = ps_z.tile([128, 512], F32, tag="zps")
        nc.tensor.transpose(p_et[:K, :BL], t_e, t_ident)
        t_eT = cpool.tile([K, BL], F16)
        nc.scalar.copy(t_eT, p_et[:K, :BL])

        # diag quads: [diag(e_{4q}) .. diag(e_{4q+3})], split DVE/ACT
        t_diag = cpool.tile([128, 2 * 512], F16)
        for k in range(K):
            dst = t_diag[:, k * 128:(k + 1) * 128]
            sc = t_e[:, k:k + 1]
            if k % 2 == 0:
                nc.vector.tensor_scalar_mul(dst, t_ident, sc)
            else:
                nc.scalar.activation(dst, t_ident, AFT.Copy, scale=sc)

        for k in range(K):
            expert_mms(k)
            expert_copy(k)

        # ---- combines, column-split into two half-width PSUM groups -------
        # The left half (output cols 0:256) stops K*IC matmuls before the
        # right half, so the boundary ELU + the next layer's z-prep pipeline
        # under the right half's matmuls instead of serializing after them.
        # Each half lives in its own full PSUM bank ([:, 0:256] of a [BL, D]
        # tile) so the ELU of one half never contends with accumulation or
        # reads of the other.
        def _elu_half(t_e, t_r, ydst, po, h):
            sl = slice(h * 256, (h + 1) * 256)
            nc.scalar.activation(
                t_e[:, sl], po[:, 0:256], AFT.Exp, bias=0.0, scale=t_rcp
            )
            nc.vector.tensor_scalar(
                t_r[:, sl], po[:, 0:256], t_rcp, 0.0, op0=ALU.mult, op1=ALU.max
            )
            nc.vector.scalar_tensor_tensor(
                ydst[:, sl], in0=t_e[:, sl], scalar=1.0, in1=t_r[:, sl],
                op0=ALU.min, op1=ALU.add,
            )

        def _l0_mm(po, k, h, start=False, stop=False):
            cs = slice(h * 256, h * 256 + 256)
            if k < 0:
                nc.tensor.matmul(
                    po[:, 0:256], lhsT=t_eT, rhs=t_bias[:, 0:D][:, cs],
                    start=True, stop=False,
                )
            else:
                nc.tensor.matmul(
                    po[:, 0:256],
                    lhsT=t_diag[:, k * 128:(k + 1) * 128],
                    rhs=t_pe[:, k * 512:(k + 1) * 512][:, cs],
                    start=False,
                    stop=stop,
                )

        def _combine_half(l, t_z, po, h, stop_last=True):
            """bias + two ic-sweeps (ic 0,1 then ic 2,3) so the first matmuls
            only need the z chunks made from the left half of y."""
            cs = slice(h * 256, h * 256 + 256)
            nc.tensor.matmul(
                po[:, 0:256], lhsT=t_eT,
                rhs=t_bias[:, l * D:(l + 1) * D][:, cs],
                start=True, stop=False,
            )
            for ics in ((0, 1), (2, 3)):
                for k in range(K):
                    q, kq = divmod(k, 4)
                    for ic in ics:
                        last = ics == (2, 3) and k == K - 1 and ic == 3
                        nc.tensor.matmul(
                            po[:, 0:256],
                            lhsT=t_z[:, q * 2048 + ic * 512 + kq * 128:
                                     q * 2048 + ic * 512 + (kq + 1) * 128],
                            rhs=t_w[:, l * LW + k * 2048 + ic * 512:
                                    l * LW + k * 2048 + (ic + 1) * 512][:, cs],
                            start=False,
                            stop=(stop_last and last),
                        )

        def _z_prep(y_src):
            # z_k.T = y.T * e[:,k], 4 experts per matmul; order brings q0's
            # ic0/1 (left-y, consumed first by the k-major combine) earliest.
            t_z = zpool.tile([128, K * D], F16, tag="z")
            order = [(0, 0), (0, 1), (1, 0), (1, 1), (0, 2), (0, 3), (1, 2), (1, 3)]
            for n, (q, ic) in enumerate(order):
                p_z = ps_z.tile([128, 512], F32, tag="zps")
                nc.tensor.matmul(
                    p_z,
                    lhsT=y_src[:, ic * 128:(ic + 1) * 128],
                    rhs=t_diag[:, q * 512:(q + 1) * 512],
                    start=True,
                    stop=True,
                )
                dst = t_z[:, q * 2048 + ic * 512:q * 2048 + (ic + 1) * 512]
                if n % 2 == 0:
                    nc.vector.tensor_copy(out=dst, in_=p_z)
                else:
                    nc.scalar.copy(dst, p_z)
            return t_z

        # layer 0: bias then per-k (L,R) matmul pairs — both halves chase the
        # expert drains together, so both stop right after expert 7 drains
        p_oL = ps_out.tile([BL, D], F32, tag="out")
        p_oR = ps_out.tile([BL, D], F32, tag="out")
        y = ypool.tile([BL, D], F16, tag="y")
        t_e0 = tpool.tile([BL, D], F32, tag="elu_e")
        t_r0 = tpool.tile([BL, D], F32, tag="elu_r")
        _l0_mm(p_oL, -1, 0)
        _l0_mm(p_oR, -1, 1)
        for k in range(K):
            _l0_mm(p_oL, k, 0, stop=(k == K - 1))
            _l0_mm(p_oR, k, 1, stop=(k == K - 1))
        _elu_half(t_e0, t_r0, y, p_oL, 0)
        _elu_half(t_e0, t_r0, y, p_oR, 1)

        # layers 1, 2: all-L (stop) -> ELU-L under the R half -> all-R (stop);
        # the next layer's z-prep consumes y-L chunks right after R stops
        for l in range(1, 3):
            t_z = _z_prep(y)
            p_oL = ps_out.tile([BL, D], F32, tag="out")
            p_oR = ps_out.tile([BL, D], F32, tag="out")
            _combine_half(l, t_z, p_oL, 0)
            if l < 2:
                y_next = ypool.tile([BL, D], F16, tag="y")
                t_e = tpool.tile([BL, D], F32, tag="elu_e")
                t_r = tpool.tile([BL, D], F32, tag="elu_r")
                _elu_half(t_e, t_r, y_next, p_oL, 0)
                _combine_half(l, t_z, p_oR, 1)
                _elu_half(t_e, t_r, y_next, p_oR, 1)
                y = y_next
            else:
                # drain + DMA each output half as soon as its bank stops:
                # the left half's copy + DMA run under the right half's mms
                t_out = ypool.tile([BL, D], F32, tag="yout")
                nc.scalar.activation(
                    t_out[:, 0:256], p_oL[:, 0:256], AFT.Copy, scale=t_rcp
                )
                nc.sync.dma_start(out=out_ap[:, 0:256], in_=t_out[:, 0:256])
                _combine_half(l, t_z, p_oR, 1)
                nc.vector.tensor_scalar(
                    t_out[:, 256:512], p_oR[:, 0:256], t_rcp, 0.0,
                    op0=ALU.mult, op1=ALU.bypass,
                )
                nc.sync.dma_start(out=out_ap[:, 256:512], in_=t_out[:, 256:512])


def _elu_shift_halves(nc, tpool, ydst, p_o, rcp):
    """ydst = elu(p_o * rcp) + 1 in two half-width pieces so the left half is
    ready ~1.1us after the PSUM stops (vs ~2.2us full-width serial) and the
    engines pipeline: exp on Scalar, relu+combine on Vector."""
    t_e = tpool.tile([BL, D], F32, tag="elu_e")
    t_r = tpool.tile([BL, D], F32, tag="elu_r")
    for h in range(2):
        sl = slice(h * 256, (h + 1) * 256)
        nc.scalar.activation(t_e[:, sl], p_o[:, sl], AFT.Exp, bias=0.0, scale=rcp)
        nc.vector.tensor_scalar(
            t_r[:, sl], p_o[:, sl], rcp, 0.0, op0=ALU.mult, op1=ALU.max
        )
        nc.vector.scalar_tensor_tensor(
            ydst[:, sl], in0=t_e[:, sl], scalar=1.0, in1=t_r[:, sl],
            op0=ALU.min, op1=ALU.add,
        )


def _elu1(nc, tpool, out, pre, bias):
    """out = elu(pre + bias) + 1 = relu(x) + min(exp(x), 1); x = pre + bias.
    exp on Scalar, relu on Vector so the two run in parallel."""
    shape = [pre.partition_size(), pre.free_size()]
    t_e = tpool.tile(shape, F32, tag="elu_e")
    nc.scalar.activation(t_e, pre, AFT.Exp, bias=bias, scale=1.0)
    t_r = tpool.tile(shape, F32, tag="elu_r")
    nc.vector.tensor_scalar(t_r, pre, bias, 0.0, op0=ALU.add, op1=ALU.max)
    nc.vector.scalar_tensor_tensor(
        out, in0=t_e, scalar=1.0, in1=t_r, op0=ALU.min, op1=ALU.add
    )


def _prep_host(x, phase, gw0, gb0, gw1, gb1, gw2, gb2, W0, b0, W1, b1, W2, b2):
    """Host-side packing. Returns per-core input maps."""
    f32 = np.float32

    # weights blob: [3, 8, 128, 2048]; [l, k, p, ic*512 + o] = W_l[k, ic*128+p, o]
    W = np.stack([W0, W1, W2]).astype(f32)  # [3, 8, 512, 512]
    Wb = (
        W.reshape(3, K, IC, 128, D)
        .transpose(0, 1, 3, 2, 4)
        .reshape(3, K, 128, IC * D)
        .astype(np.float16)
    )
    # +1-shift corrections: layer l>0 consumes y'+1, gate layers 1,2 consume h'+1
    b0a = np.asarray(b0, f32)
    b1a = np.asarray(b1, f32) - np.asarray(W1, f32).sum(axis=1)
    b2a = np.asarray(b2, f32) - np.asarray(W2, f32).sum(axis=1)
    eb = np.concatenate([b0a, b1a, b2a], axis=1).astype(np.float16)  # [8, 1536]
    gb1a = np.asarray(gb1, f32) - np.asarray(gw1, f32).sum(axis=0)
    gb2a = np.asarray(gb2, f32) - np.asarray(gw2, f32).sum(axis=0)

    # packed fp32 const blob [128, 530]:
    #   0:128 ident | 128:256 gw1 | 256:384 gw0 (rows 0:64)
    #   | 384:512 ph.T (rows 0:64) | 512:520 gw2 | 520 gb0 | 521 gb1
    #   | 522:530 gb2 (row 0)
    c32 = np.zeros((128, 530), f32)
    c32[:, 0:128] = np.eye(128, dtype=f32)
    c32[:, 128:256] = np.asarray(gw1, f32)
    c32[0:PHASE, 256:384] = np.asarray(gw0, f32)
    c32[:, 512:520] = np.asarray(gw2, f32)
    c32[:, 520] = np.asarray(gb0, f32)
    c32[:, 521] = gb1a
    c32[0, 522:530] = gb2a

    per_core = []
    for c in range(NCORES):
        sl = slice(c * BL, (c + 1) * BL)
        cc32 = c32.copy()
        cc32[0:PHASE, 384:512] = np.asarray(phase[sl], f32).T
        # c16 [128, 512]: x.T chunks ([p, ic*128+b] = x[b, ic*128+p])
        xs = np.asarray(x[sl]).astype(np.float16)
        c16 = xs.T.reshape(IC, 128, BL).transpose(1, 0, 2).reshape(128, IC * BL)
        per_core.append(
            {
                "c32": np.ascontiguousarray(cc32),
                "c16": np.ascontiguousarray(c16),
                "cb": np.ascontiguousarray(eb),
                "W": Wb,
            }
        )
    return per_core


def _declare_dram(nc):
    f32 = mybir.dt.float32
    ins = {
        "c32": nc.dram_tensor("c32", [128, 530], f32, kind="ExternalInput").ap(),
        "c16": nc.dram_tensor("c16", [128, D], F16, kind="ExternalInput").ap(),
        "cb": nc.dram_tensor("cb", [K, 3 * D], F16, kind="ExternalInput").ap(),
        "W": nc.dram_tensor("W", [3, K, 128, IC * D], F16, kind="ExternalInput").ap(),
    }
    out = nc.dram_tensor("out", [BL, D], f32, kind="ExternalOutput").ap()
    return ins, out


_CACHED = None


def _build():
    global _CACHED
    if _CACHED is None:
        nc = bacc.Bacc(
            "TRN2", target_bir_lowering=False, debug=False, num_devices=NCORES
        )
        ins, out = _declare_dram(nc)
        with tile.TileContext(nc) as tc:
            emit_moe(tc, out, ins)
        nc.compile()
        _CACHED = nc
    return _CACHED


def kernel(**inputs) -> np.ndarray:
    from concourse.bass_utils import run_bass_kernel_spmd

    per_core = _prep_host(**inputs)
    nc = _build()
    res = run_bass_kernel_spmd(nc, per_core, core_ids=list(range(NCORES)))
    return np.concatenate([r["out"] for r in res.results], axis=0)


if __name__ == "__main__":
    import reference

    inp = {k: np.asarray(v) for k, v in reference.setup_inputs().items()}
    got = kernel(**inp)
    exp = np.asarray(reference.reference(**inp))
    err = np.abs(got - exp).max() / np.abs(exp).max()
    print("Relative error:", err)

